# revision 13
# baseline (speedup 1.0000x reference)
"""Trainium2 Bass kernel for a differentiable addressing head (NTM-style).

Computes, for each batch b:
    key   = cs @ Wk;  beta = softplus(cs@Wb+bb)+1;  gate = sigmoid(cs@Wg+bg)
    shift = softmax(cs@Ws+bs);  gamma = softplus(cs@Wgam+bgam)+1
    sim   = (key . mem[n]) / (|key||mem[n]| + eps)
    cw    = softmax(beta * sim);  g = gate*cw + (1-gate)*pw
    sh    = circular_conv(g, shift);  w = (sh+1e-8)^gamma / (sum + eps)

Sharding: data-parallel over batch across 8 cores (8 batches/core).

Heavy pipeline per batch b (stripe = full row [128(D), 8192(N)] bf16 with
16 KB-contiguous DMA descriptors per partition, fetched in halves):
  DMA(b) -> dots(b) on PE -> squares(b) OUT-OF-PLACE (ACT/DVE/GPSIMD
  split) -> norms(b) on PE two batches behind dots.

The light phase for each 2-batch group is cut into 6 stages (A..F) that
are pipelined across heavy-loop iterations: each stage's PE micro-matmul
only depends on ACT/DVE work from >= 1 iteration earlier, so the
in-order PE never stalls on the light chain. Per-batch scalars live at
partitions P(b)=32*(b//2)+(b%2) so every tiny fp32 matmul has 32-aligned
tile positions.

Light layout: row p = 16b + t holds n in [512t, 512(t+1)) of batch b.

Self-contained: hardcodes shapes B=64, N=8192, D=128, C=256.
"""

import os
import sys

import numpy as np

for _p in ("/opt/trn_rl_repo", "/opt/pypackages"):
    if _p not in sys.path and os.path.isdir(_p):
        sys.path.insert(0, _p)

import concourse.bacc as bacc
import concourse.bass as bass
import concourse.tile as tile
from concourse import mybir
from concourse.bass_utils import run_bass_kernel_spmd

F32 = mybir.dt.float32
BF16 = mybir.dt.bfloat16
AF = mybir.ActivationFunctionType
OP = mybir.AluOpType

B, N, D, C = 64, 8192, 128, 256
NCORES = 8
BL = B // NCORES          # batches per core = 8
NW = 16                   # 512-wide windows per batch row
W = N // NW               # 512, window width (= light-tile free dim)
EPS = 1e-8
NG = BL // 2              # 2-batch light groups

# square-engine split (multiples of W): [0:SA]=ACT, [SA:SV]=DVE, [SV:N]=GP
SA = 3072
SV = 7168
HALF = N // 2

# packed external-input layout (columns of ext[128, EXTW]):
EXT_CS0, EXT_CS1 = 0, 128
EXT_WK0, EXT_WK1 = 256, 384
EXT_WC0, EXT_WC1 = 512, 518
EXT_B6 = 524
EXTW = 530
EXT_STAGED = 524          # cols staged for fp32 matmuls (csT/Wk/Wc)

# packed const layout (columns of cpk[128, CPKW]):
CP_IND2 = 0
CP_INDT2 = 2
CP_PN = 130
CP_PP = 258
CP_IP = 386
CP_BB = 514
CP_ONE = 642
CP_STRIP = 643
CP_EPS = 706
CPKW = 707
CP_STAGED = 643           # ind2..ones_col staged f32 (strip staged bf16)

_NC = None
PROFILE = False
LAST_RESULTS = None


def _pos(b):
    return 32 * (b // 2) + (b % 2)


def _consts():
    cpk = np.zeros((128, CPKW), np.float32)
    for p in range(128):
        cpk[p, CP_IND2 + (p % 32) // 16] = 1.0
    for j in range(4):
        for a in range(2):
            for t in range(16):
                cpk[32 * j + a, CP_INDT2 + 32 * j + 16 * a + t] = 1.0
    for m in range(128):
        bb, t = m // 16, m % 16
        cpk[16 * bb + (t + 1) % 16, CP_PN + m] = 1.0
        cpk[16 * bb + (t - 1) % 16, CP_PP + m] = 1.0
    for j in range(4):
        for a in range(2):
            for t in range(16):
                cpk[32 * j + 16 * a + t, CP_IP + 32 * j + a] = 1.0
    for p in range(128):
        for m in range(128):
            if p // 16 == m // 16:
                cpk[p, CP_BB + m] = 1.0
    cpk[:, CP_ONE] = 1.0
    cpk[:, CP_STRIP + 31] = 1.0
    cpk[:, CP_EPS] = EPS
    return cpk


def _patch_act_tables():
    """Keep exp+ln+square+copy in ONE ACT table set (a swap costs 1.3us)."""
    import concourse.hw_specs as hw_specs  # noqa: F401

    if getattr(bacc, "_act_tables_patched", False):
        return
    orig = bacc.get_activation_tables

    def filtered(module_arch):
        t = orig(module_arch)
        pref = "natural_log_exp_and_others"
        if pref in t:
            mine = {
                AF.Exp, AF.Ln, AF.Square, AF.Copy, AF.Identity, AF.MemsetZero
            } & t[pref]
            for k in t:
                if k != pref:
                    t[k] = t[k] - mine
        return t

    bacc.get_activation_tables = filtered
    bacc._act_tables_patched = True


def build_nc():
    _patch_act_tables()
    nc = bacc.Bacc()

    memT_d = nc.dram_tensor("memT", [BL, D, N], BF16, kind="ExternalInput")
    ext_d = nc.dram_tensor("ext", [128, EXTW], F32, kind="ExternalInput")
    pw_d = nc.dram_tensor("pw", [BL, N], F32, kind="ExternalInput")
    out_d = nc.dram_tensor("out", [BL, N], F32, kind="ExternalOutput")
    cpk_c = nc.inline_tensor(_consts(), "cpk_c")

    with tile.TileContext(nc) as tc:
        with (
            tc.tile_pool(name="const", bufs=1) as cp,
            tc.tile_pool(name="mem", bufs=6) as memp,
            tc.tile_pool(name="sq", bufs=3) as sqp,
            tc.tile_pool(name="light", bufs=1) as lp,
            tc.tile_pool(name="psmm", bufs=2, space="PSUM") as psA,
            tc.tile_pool(name="pstiny", bufs=2, space="PSUM") as psB,
        ):
            # strips memset depends on nothing: issue before everything
            strips = cp.tile([128, BL, 63], BF16)
            nc.vector.memset(strips[:], 0.0)

            # ---- packed input/const DMAs + first stripe quarters ----
            ext_raw = cp.tile([128, EXTW], F32, name="ext_raw")
            nc.scalar.dma_start(ext_raw[:], ext_d[:])
            memT_ap = memT_d[:]
            raw_t = []
            st0 = memp.tile([128, N], BF16, tag="raw", name="raw_0")
            raw_t.append(st0)
            Q = N // 4
            for q in range(4):
                nc.sync.dma_start(st0[:, q * Q : (q + 1) * Q],
                                  memT_ap[0][:, q * Q : (q + 1) * Q])
            st1 = memp.tile([128, N], BF16, tag="raw", name="raw_1")
            raw_t.append(st1)
            nc.scalar.dma_start(st1[:, 0:HALF], memT_ap[1][:, 0:HALF])
            cpk_raw = cp.tile([128, CPKW], F32, name="cpk_raw")
            nc.scalar.dma_start(cpk_raw[:], cpk_c[:])
            nc.scalar.dma_start(st1[:, HALF:N], memT_ap[1][:, HALF:N])
            pw_sb = cp.tile([128, W], F32)
            nc.scalar.dma_start(pw_sb[:], pw_d[:].rearrange("b (q f) -> (b q) f", f=W))
            for b in range(2, BL):
                st = memp.tile([128, N], BF16, tag="raw", name=f"raw_{b}")
                eng = nc.sync if b % 2 == 0 else nc.scalar
                eng.dma_start(st[:, 0:HALF], memT_ap[b][:, 0:HALF])
                eng.dma_start(st[:, HALF:N], memT_ap[b][:, HALF:N])
                raw_t.append(st)

            # packed tiles are each written by ONE DMA instruction, so a
            # matmul reading any slices of one pack carries one DMA tick;
            # absorbers below make PE observe each pack's tick once.
            ones_strip = cp.tile([128, 63], BF16, name="strip_g")
            nc.vector.tensor_copy(ones_strip[:], cpk_raw[:, CP_STRIP : CP_STRIP + 63])

            csT0 = ext_raw[:, EXT_CS0 : EXT_CS0 + 128]
            csT1 = ext_raw[:, EXT_CS1 : EXT_CS1 + 128]
            Wk0 = ext_raw[:, EXT_WK0 : EXT_WK0 + 128]
            Wk1 = ext_raw[:, EXT_WK1 : EXT_WK1 + 128]
            Wc0 = ext_raw[:, EXT_WC0 : EXT_WC0 + 6]
            Wc1 = ext_raw[:, EXT_WC1 : EXT_WC1 + 6]
            bias6 = ext_raw[:, EXT_B6 : EXT_B6 + 6]
            ind2 = cpk_raw[:, CP_IND2 : CP_IND2 + 2]
            indT2 = cpk_raw[:, CP_INDT2 : CP_INDT2 + 128]
            pnext = cpk_raw[:, CP_PN : CP_PN + 128]
            pprev = cpk_raw[:, CP_PP : CP_PP + 128]
            indP = cpk_raw[:, CP_IP : CP_IP + 128]
            ind_bb = cpk_raw[:, CP_BB : CP_BB + 128]
            ones_col = cpk_raw[:, CP_ONE : CP_ONE + 1]

            # ---- projections ----
            key_ps = psB.tile([128, 128], F32, tag="keyps", bufs=1)
            nc.tensor.matmul(key_ps[:], lhsT=Wk0, rhs=csT0, start=True, stop=False)
            nc.tensor.matmul(key_ps[:], lhsT=Wk1, rhs=csT1, start=False, stop=True)
            for b in range(BL):
                nc.vector.tensor_copy(
                    strips[:, b, 31:32], key_ps[:, _pos(b) : _pos(b) + 1]
                )

            proj_ps = psB.tile([128, 6], F32, tag="tiny", bufs=3)
            nc.tensor.matmul(proj_ps[:], lhsT=csT0, rhs=Wc0, start=True, stop=False)
            nc.tensor.matmul(proj_ps[:], lhsT=csT1, rhs=Wc1, start=False, stop=True)
            proj = lp.tile([128, 6], F32)
            nc.vector.tensor_add(proj[:], proj_ps[:], bias6)

            # absorbers: make PE observe the cpk-DMA and DVE-strip ticks once
            absorb = psB.tile([128, 8], F32, tag="tiny", bufs=3, name="absorb")
            nc.tensor.matmul(absorb[:, 0:1], lhsT=indT2, rhs=ones_col,
                             start=True, stop=True, skip_group_check=True)
            nc.tensor.matmul(absorb[0:63, 1:2], lhsT=ones_strip[:],
                             rhs=ones_strip[:, 31:32],
                             start=True, stop=True, skip_group_check=True)

            # |key|^2 -> F_kn2 broadcast to window rows
            kq = lp.tile([128, 128], F32)
            nc.scalar.activation(kq[:], key_ps[:], AF.Square)
            kn2_ps = psB.tile([128, 1], F32, tag="tiny", bufs=3)
            nc.tensor.matmul(kn2_ps[:], lhsT=kq[:], rhs=ones_col, start=True, stop=True)
            kn2 = lp.tile([128, 1], F32)
            nc.vector.tensor_copy(kn2[:], kn2_ps[:])
            fkn_ps = psB.tile([128, 1], F32, tag="tiny", bufs=3)
            nc.tensor.matmul(fkn_ps[:], lhsT=indT2, rhs=kn2[:], start=True, stop=True)
            F_kn2 = lp.tile([128, 1], F32)
            nc.vector.tensor_copy(F_kn2[:], fkn_ps[:])

            # ---- per-batch scalars at rows P(b) ----
            scal = lp.tile([128, 7], F32)
            eb = lp.tile([128, 1], F32)
            nc.scalar.activation(eb[:], proj[:, 0:1], AF.Exp)
            sp_b = lp.tile([128, 1], F32)
            nc.scalar.activation(sp_b[:], eb[:], AF.Ln, bias=1.0)
            nc.vector.tensor_scalar_add(scal[:, 0:1], sp_b[:], 1.0)
            eg = lp.tile([128, 1], F32)
            nc.scalar.activation(eg[:], proj[:, 1:2], AF.Exp, scale=-1.0)
            dg = lp.tile([128, 1], F32)
            nc.vector.tensor_scalar_add(dg[:], eg[:], 1.0)
            gate = lp.tile([128, 1], F32)
            nc.vector.reciprocal(gate[:], dg[:])
            nc.vector.tensor_scalar(
                scal[:, 1:2], gate[:], -1.0, 1.0, op0=OP.mult, op1=OP.add
            )
            e3 = lp.tile([128, 3], F32)
            nc.scalar.activation(e3[:], proj[:, 2:5], AF.Exp)
            ssum = lp.tile([128, 1], F32)
            nc.vector.reduce_sum(ssum[:], e3[:], axis=mybir.AxisListType.X)
            rssum = lp.tile([128, 1], F32)
            nc.vector.reciprocal(rssum[:], ssum[:])
            sh3 = lp.tile([128, 3], F32)
            nc.scalar.mul(sh3[:], e3[:], rssum[:])
            nc.vector.tensor_copy(scal[:, 2:5], sh3[:])
            egm = lp.tile([128, 1], F32)
            nc.scalar.activation(egm[:], proj[:, 5:6], AF.Exp)
            sp_g = lp.tile([128, 1], F32)
            nc.scalar.activation(sp_g[:], egm[:], AF.Ln, bias=1.0)
            nc.vector.tensor_scalar_add(scal[:, 5:6], sp_g[:], 1.0)
            nc.vector.tensor_copy(scal[:, 6:7], gate[:])
            FB_ps = psB.tile([128, 7], F32, tag="tiny", bufs=3)
            nc.tensor.matmul(FB_ps[:], lhsT=indT2, rhs=scal[:], start=True, stop=True)
            FB = lp.tile([128, 7], F32)
            nc.vector.tensor_copy(FB[:], FB_ps[:])
            F_beta = FB[:, 0:1]
            F_g1 = FB[:, 1:2]
            F_s0 = FB[:, 2:3]
            F_s1 = FB[:, 3:4]
            F_s2 = FB[:, 4:5]
            F_gamma = FB[:, 5:6]
            F_gate = FB[:, 6:7]

            t4_all = lp.tile([128, W], F32)
            nc.vector.tensor_scalar_mul(t4_all[:], pw_sb[:], F_g1)

            # ---- light tiles ----
            Lv = lp.tile([128, W], F32)
            y1 = lp.tile([128, W], F32)
            sim = lp.tile([128, W], F32)
            E = lp.tile([128, W], F32)
            G = lp.tile([128, W], F32)
            SH = lp.tile([128, W], F32)
            Lg = lp.tile([128, W], F32)
            P2 = lp.tile([128, W], F32)
            outsb = lp.tile([128, W], F32)
            rs1 = lp.tile([128, 1], F32)
            rs2 = lp.tile([128, 1], F32)
            rS = lp.tile([128, 1], F32)
            gs_all = lp.tile([128, 1], F32)
            S2a = lp.tile([128, 1], F32)
            r2a = lp.tile([128, 1], F32)
            F_gs = lp.tile([128, 1], F32)
            F_r2 = lp.tile([128, 1], F32)
            bl_sb = lp.tile([128, 1], F32)
            br_sb = lp.tile([128, 1], F32)

            dotP = psA.tile([128, W], F32, tag="dotP", bufs=1, name="dotP")
            nrmP = psA.tile([128, W], F32, tag="nrmP", bufs=1, name="nrmP")
            sq_t = {}

            def dots(b):
                j = b // 2
                rows = slice(32 * j, 32 * j + 32)
                st = raw_t[b]
                for t in range(NW):
                    c = NW * (b % 2) + t
                    nc.tensor.matmul(
                        dotP[rows, :],
                        lhsT=strips[:, b, 31 - c : 63 - c],
                        rhs=st[:, t * W : (t + 1) * W],
                        start=(b % 2 == 0) and (t == 0),
                        stop=(b % 2 == 1) and (t == NW - 1),
                        skip_group_check=True,
                        tile_position=(0, 32 * j),
                    )

            def squares(b):
                sq = sqp.tile([128, N], BF16, tag="sq", name=f"sq_{b}")
                st = raw_t[b]
                nc.scalar.activation(sq[:, 0:SA], st[:, 0:SA], AF.Square)
                nc.vector.tensor_mul(sq[:, SA:HALF], st[:, SA:HALF], st[:, SA:HALF])
                nc.vector.tensor_mul(sq[:, HALF:SV], st[:, HALF:SV], st[:, HALF:SV])
                nc.gpsimd.tensor_mul(sq[:, SV:N], st[:, SV:N], st[:, SV:N])
                sq_t[b] = sq

            def norms(b):
                j = b // 2
                rows = slice(32 * j, 32 * j + 32)
                sq = sq_t[b]
                for t in range(NW):
                    c = NW * (b % 2) + t
                    nc.tensor.matmul(
                        nrmP[rows, :],
                        lhsT=ones_strip[:, 31 - c : 63 - c],
                        rhs=sq[:, t * W : (t + 1) * W],
                        start=(b % 2 == 0) and (t == 0),
                        stop=(b % 2 == 1) and (t == NW - 1),
                        skip_group_check=True,
                        tile_position=(0, 32 * j),
                    )

            def stageA(j):
                R = slice(32 * j, 32 * j + 32)
                nc.scalar.activation(Lv[R, :], nrmP[R, :], AF.Ln,
                                     scale=F_kn2[R, :])
                nc.scalar.activation(y1[R, :], Lv[R, :], AF.Exp, scale=-0.5)
                nc.vector.tensor_mul(sim[R, :], dotP[R, :], y1[R, :])
                nc.scalar.activation(E[R, :], sim[R, :], AF.Exp,
                                     scale=F_beta[R, :], accum_out=rs1[R, :])

            for it in range(BL + 1):
                if it < BL:
                    dots(it)
                    squares(it)
                if it >= 1:
                    norms(it - 1)
                    if (it - 1) % 2 == 1:
                        stageA((it - 1) // 2)

            # ---- tail: batch-sum-broadcast via block-diag ind_bb in ONE
            # matmul, then full-width chain ----
            sps = psB.tile([128, 1], F32, tag="tiny", bufs=3, name="sps")
            nc.tensor.matmul(sps[:], lhsT=ind_bb, rhs=rs1[:],
                             start=True, stop=True, skip_group_check=True)
            nc.vector.reciprocal(rS[:], sps[:])
            nc.vector.tensor_mul(F_gs[:], F_gate, rS[:])
            nc.vector.scalar_tensor_tensor(
                G[:], E[:], F_gs[:], t4_all[:], op0=OP.mult, op1=OP.add
            )
            blps = psB.tile([128, 1], F32, tag="tiny", bufs=3, name="blps")
            nc.tensor.matmul(blps[:], lhsT=pnext, rhs=G[:, 0:1],
                             start=True, stop=True, skip_group_check=True)
            brps = psB.tile([128, 1], F32, tag="tiny", bufs=3, name="brps")
            nc.tensor.matmul(brps[:], lhsT=pprev, rhs=G[:, W - 1 : W],
                             start=True, stop=True, skip_group_check=True)
            nc.vector.tensor_copy(bl_sb[:], blps[:])
            nc.vector.tensor_copy(br_sb[:], brps[:])
            nc.scalar.mul(SH[:], G[:], F_s1)
            nc.vector.scalar_tensor_tensor(
                SH[:, 0 : W - 1], G[:, 1:W], F_s0, SH[:, 0 : W - 1],
                op0=OP.mult, op1=OP.add,
            )
            nc.vector.scalar_tensor_tensor(
                SH[:, 1:W], G[:, 0 : W - 1], F_s2, SH[:, 1:W],
                op0=OP.mult, op1=OP.add,
            )
            nc.vector.scalar_tensor_tensor(
                SH[:, W - 1 : W], bl_sb[:], F_s0, SH[:, W - 1 : W],
                op0=OP.mult, op1=OP.add,
            )
            nc.vector.scalar_tensor_tensor(
                SH[:, 0:1], br_sb[:], F_s2, SH[:, 0:1],
                op0=OP.mult, op1=OP.add,
            )
            nc.scalar.activation(Lg[:], SH[:], AF.Ln,
                                 bias=cpk_raw[:, CP_EPS : CP_EPS + 1])
            nc.scalar.activation(P2[:], Lg[:], AF.Exp,
                                 scale=F_gamma, accum_out=rs2[:])
            s2ps = psB.tile([128, 1], F32, tag="tiny", bufs=3, name="s2ps")
            nc.tensor.matmul(s2ps[:], lhsT=ind_bb, rhs=rs2[:],
                             start=True, stop=True, skip_group_check=True)
            nc.vector.tensor_scalar_add(S2a[:], s2ps[:], EPS)
            nc.vector.reciprocal(F_r2[:], S2a[:])
            nc.scalar.mul(outsb[:], P2[:], F_r2)
            nc.sync.dma_start(
                out_d[:].rearrange("b (q f) -> (b q) f", f=W), outsb[:]
            )
    nc.compile()
    return nc


def _get_nc():
    global _NC
    if _NC is None:
        _NC = build_nc()
    return _NC


def _enable_profiling():
    """Install the axon NTFF profile hook; the agent image lacks
    antenv.axon_hooks, so shim it and register the ctypes-based hook."""
    import types

    import concourse.bass_utils as bu

    bu.upload_artifacts = lambda tmpdir: tmpdir
    try:
        from antenv.axon_hooks import get_axon_ntff_profile_hook  # noqa: F401

        return
    except ImportError:
        pass
    import antenv

    mod = types.ModuleType("antenv.axon_hooks")
    _holder = {}
    mod.set_axon_ntff_profile_hook = lambda h: _holder.__setitem__("h", h)
    mod.get_axon_ntff_profile_hook = lambda: _holder.get("h")
    sys.modules["antenv.axon_hooks"] = mod
    antenv.axon_hooks = mod
    from trn_agent_boot.trn_boot import _ntff_profile_via_ctypes

    mod.set_axon_ntff_profile_hook(
        _ntff_profile_via_ctypes("/opt/axon/libaxon_pjrt.so")
    )


def kernel(**inputs):
    global LAST_RESULTS
    mem = np.ascontiguousarray(np.asarray(inputs["memory"], dtype=np.float32))
    cs = np.ascontiguousarray(np.asarray(inputs["controller_state"], dtype=np.float32))
    pw = np.ascontiguousarray(np.asarray(inputs["previous_weights"], dtype=np.float32))
    Wk = np.ascontiguousarray(np.asarray(inputs["Wk"], dtype=np.float32))
    Wcat = np.concatenate(
        [
            np.asarray(inputs["Wb"], np.float32),
            np.asarray(inputs["Wg"], np.float32),
            np.asarray(inputs["Ws"], np.float32),
            np.asarray(inputs["Wgam"], np.float32),
        ],
        axis=1,
    )
    brow = np.concatenate(
        [
            np.asarray(inputs["bb"], np.float32),
            np.asarray(inputs["bg"], np.float32),
            np.asarray(inputs["bs"], np.float32),
            np.asarray(inputs["bgam"], np.float32),
        ]
    )
    pos = np.array([_pos(b) for b in range(BL)])

    # shard: core c gets batches [c*BL, (c+1)*BL); memory pre-transposed
    memT = np.ascontiguousarray(
        mem.reshape(NCORES, BL, N, D).transpose(0, 1, 3, 2)
    )
    import ml_dtypes
    memT = memT.astype(ml_dtypes.bfloat16)
    csT_all = cs.reshape(NCORES, BL, C).transpose(0, 2, 1)  # (8, C, BL)
    pw_sh = pw.reshape(NCORES, BL, N)

    # packed external input: csT0|csT1|Wk0|Wk1|Wc0|Wc1|bias6
    ext = np.zeros((NCORES, 128, EXTW), np.float32)
    ext[:, :, EXT_WK0 : EXT_WK0 + 128] = Wk[0:128, :][None]
    ext[:, :, EXT_WK1 : EXT_WK1 + 128] = Wk[128:256, :][None]
    ext[:, :, EXT_WC0 : EXT_WC0 + 6] = Wcat[0:128, :][None]
    ext[:, :, EXT_WC1 : EXT_WC1 + 6] = Wcat[128:256, :][None]
    ext[:, pos, EXT_B6 : EXT_B6 + 6] = brow[None, None, :]
    for bi in range(BL):
        ext[:, :, EXT_CS0 + pos[bi]] = csT_all[:, 0:128, bi]
        ext[:, :, EXT_CS1 + pos[bi]] = csT_all[:, 128:256, bi]

    in_maps = [
        {
            "memT": memT[c],
            "ext": np.ascontiguousarray(ext[c]),
            "pw": np.ascontiguousarray(pw_sh[c]),
        }
        for c in range(NCORES)
    ]
    nc = _get_nc()
    if PROFILE:
        _enable_profiling()
    res = run_bass_kernel_spmd(nc, in_maps, list(range(NCORES)), trace=PROFILE)
    LAST_RESULTS = res
    out = np.concatenate([r["out"] for r in res.results], axis=0)
    return out.astype(np.float32)


# revision 14
# speedup vs baseline: 1.3940x; 1.3940x over previous
"""Trainium2 Bass kernel for a differentiable addressing head (NTM-style).

Computes, for each batch b:
    key   = cs @ Wk;  beta = softplus(cs@Wb+bb)+1;  gate = sigmoid(cs@Wg+bg)
    shift = softmax(cs@Ws+bs);  gamma = softplus(cs@Wgam+bgam)+1
    sim   = (key . mem[n]) / (|key||mem[n]| + eps)
    cw    = softmax(beta * sim);  g = gate*cw + (1-gate)*pw
    sh    = circular_conv(g, shift);  w = (sh+1e-8)^gamma / (sum + eps)

Sharding: data-parallel over batch across 8 cores (8 batches/core).

Heavy pipeline per batch b (stripe = full row [128(D), 8192(N)] bf16 with
16 KB-contiguous DMA descriptors per partition, fetched in halves):
  DMA(b) -> dots(b) on PE -> squares(b) OUT-OF-PLACE (ACT/DVE/GPSIMD
  split) -> norms(b) on PE two batches behind dots.

The light phase for each 2-batch group is cut into 6 stages (A..F) that
are pipelined across heavy-loop iterations: each stage's PE micro-matmul
only depends on ACT/DVE work from >= 1 iteration earlier, so the
in-order PE never stalls on the light chain. Per-batch scalars live at
partitions P(b)=32*(b//2)+(b%2) so every tiny fp32 matmul has 32-aligned
tile positions.

Light layout: row p = 16b + t holds n in [512t, 512(t+1)) of batch b.

Self-contained: hardcodes shapes B=64, N=8192, D=128, C=256.
"""

import os
import sys

import numpy as np

for _p in ("/opt/trn_rl_repo", "/opt/pypackages"):
    if _p not in sys.path and os.path.isdir(_p):
        sys.path.insert(0, _p)

import concourse.bacc as bacc
import concourse.bass as bass
import concourse.tile as tile
from concourse import mybir
from concourse.bass_utils import run_bass_kernel_spmd

F32 = mybir.dt.float32
BF16 = mybir.dt.bfloat16
AF = mybir.ActivationFunctionType
OP = mybir.AluOpType

B, N, D, C = 64, 8192, 128, 256
NCORES = 8
BL = B // NCORES          # batches per core = 8
NW = 16                   # 512-wide windows per batch row
W = N // NW               # 512, window width (= light-tile free dim)
EPS = 1e-8
NG = BL // 2              # 2-batch light groups

# square-engine split (multiples of W): [0:SA]=ACT, [SA:SV]=DVE, [SV:N]=GP
SA = 3072
SV = 7168
HALF = N // 2

# packed external-input layout (columns of ext[128, EXTW]):
EXT_CS0, EXT_CS1 = 0, 128
EXT_WK0, EXT_WK1 = 256, 384
EXT_WC0, EXT_WC1 = 512, 518
EXT_B6 = 524
EXTW = 530
EXT_STAGED = 524          # cols staged for fp32 matmuls (csT/Wk/Wc)

# packed const layout (columns of cpk[128, CPKW]):
CP_IND2 = 0
CP_INDT2 = 2
CP_PN = 130
CP_PP = 258
CP_IP = 386
CP_BB = 514
CP_ONE = 642
CP_STRIP = 643
CP_EPS = 706
CPKW = 707
CP_STAGED = 643           # ind2..ones_col staged f32 (strip staged bf16)

_NC = None
PROFILE = False
LAST_RESULTS = None


def _pos(b):
    return 32 * (b // 2) + (b % 2)


def _consts():
    cpk = np.zeros((128, CPKW), np.float32)
    for p in range(128):
        cpk[p, CP_IND2 + (p % 32) // 16] = 1.0
    for j in range(4):
        for a in range(2):
            for t in range(16):
                cpk[32 * j + a, CP_INDT2 + 32 * j + 16 * a + t] = 1.0
    for m in range(128):
        bb, t = m // 16, m % 16
        cpk[16 * bb + (t + 1) % 16, CP_PN + m] = 1.0
        cpk[16 * bb + (t - 1) % 16, CP_PP + m] = 1.0
    for j in range(4):
        for a in range(2):
            for t in range(16):
                cpk[32 * j + 16 * a + t, CP_IP + 32 * j + a] = 1.0
    for p in range(128):
        for m in range(128):
            if p // 16 == m // 16:
                cpk[p, CP_BB + m] = 1.0
    cpk[:, CP_ONE] = 1.0
    cpk[:, CP_STRIP + 31] = 1.0
    cpk[:, CP_EPS] = EPS
    return cpk


def _patch_act_tables():
    """Keep exp+ln+square+copy in ONE ACT table set (a swap costs 1.3us)."""
    import concourse.hw_specs as hw_specs  # noqa: F401

    if getattr(bacc, "_act_tables_patched", False):
        return
    orig = bacc.get_activation_tables

    def filtered(module_arch):
        t = orig(module_arch)
        pref = "natural_log_exp_and_others"
        if pref in t:
            mine = {
                AF.Exp, AF.Ln, AF.Square, AF.Copy, AF.Identity, AF.MemsetZero
            } & t[pref]
            for k in t:
                if k != pref:
                    t[k] = t[k] - mine
        return t

    bacc.get_activation_tables = filtered
    bacc._act_tables_patched = True


def build_nc():
    _patch_act_tables()
    nc = bacc.Bacc()

    memT_d = nc.dram_tensor("memT", [BL, D, N], BF16, kind="ExternalInput")
    ext_d = nc.dram_tensor("ext", [128, EXTW], F32, kind="ExternalInput")
    pw_d = nc.dram_tensor("pw", [BL, N], F32, kind="ExternalInput")
    out_d = nc.dram_tensor("out", [BL, N], F32, kind="ExternalOutput")
    cpk_c = nc.inline_tensor(_consts(), "cpk_c")

    with tile.TileContext(nc) as tc:
        with (
            tc.tile_pool(name="const", bufs=1) as cp,
            tc.tile_pool(name="mem", bufs=6) as memp,
            tc.tile_pool(name="sq", bufs=3) as sqp,
            tc.tile_pool(name="light", bufs=1) as lp,
            tc.tile_pool(name="psmm", bufs=2, space="PSUM") as psA,
            tc.tile_pool(name="pstiny", bufs=2, space="PSUM") as psB,
        ):
            # strips memset depends on nothing: issue before everything
            strips = cp.tile([128, BL, 63], BF16)
            nc.vector.memset(strips[:], 0.0)

            # ---- packed input/const DMAs + first stripe quarters ----
            ext_raw = cp.tile([128, EXTW], F32, name="ext_raw")
            nc.scalar.dma_start(ext_raw[:], ext_d[:])
            memT_ap = memT_d[:]
            raw_t = []
            st0 = memp.tile([128, N], BF16, tag="raw", name="raw_0")
            raw_t.append(st0)
            Q = N // 4
            for q in range(4):
                nc.sync.dma_start(st0[:, q * Q : (q + 1) * Q],
                                  memT_ap[0][:, q * Q : (q + 1) * Q])
            st1 = memp.tile([128, N], BF16, tag="raw", name="raw_1")
            raw_t.append(st1)
            nc.sync.dma_start(st1[:, 0:HALF], memT_ap[1][:, 0:HALF])
            cpk_raw = cp.tile([128, CPKW], F32, name="cpk_raw")
            nc.scalar.dma_start(cpk_raw[:], cpk_c[:])
            nc.sync.dma_start(st1[:, HALF:N], memT_ap[1][:, HALF:N])
            pw_sb = cp.tile([128, W], F32)
            nc.scalar.dma_start(pw_sb[:], pw_d[:].rearrange("b (q f) -> (b q) f", f=W))
            for b in range(2, BL):
                st = memp.tile([128, N], BF16, tag="raw", name=f"raw_{b}")
                eng = nc.sync
                eng.dma_start(st[:, 0:HALF], memT_ap[b][:, 0:HALF])
                eng.dma_start(st[:, HALF:N], memT_ap[b][:, HALF:N])
                raw_t.append(st)

            # packed tiles are each written by ONE DMA instruction, so a
            # matmul reading any slices of one pack carries one DMA tick;
            # absorbers below make PE observe each pack's tick once.
            ones_strip = cp.tile([128, 63], BF16, name="strip_g")
            nc.vector.tensor_copy(ones_strip[:], cpk_raw[:, CP_STRIP : CP_STRIP + 63])

            csT0 = ext_raw[:, EXT_CS0 : EXT_CS0 + 128]
            csT1 = ext_raw[:, EXT_CS1 : EXT_CS1 + 128]
            Wk0 = ext_raw[:, EXT_WK0 : EXT_WK0 + 128]
            Wk1 = ext_raw[:, EXT_WK1 : EXT_WK1 + 128]
            Wc0 = ext_raw[:, EXT_WC0 : EXT_WC0 + 6]
            Wc1 = ext_raw[:, EXT_WC1 : EXT_WC1 + 6]
            bias6 = ext_raw[:, EXT_B6 : EXT_B6 + 6]
            ind2 = cpk_raw[:, CP_IND2 : CP_IND2 + 2]
            indT2 = cpk_raw[:, CP_INDT2 : CP_INDT2 + 128]
            pnext = cpk_raw[:, CP_PN : CP_PN + 128]
            pprev = cpk_raw[:, CP_PP : CP_PP + 128]
            indP = cpk_raw[:, CP_IP : CP_IP + 128]
            ind_bb = cpk_raw[:, CP_BB : CP_BB + 128]
            ones_col = cpk_raw[:, CP_ONE : CP_ONE + 1]

            # ---- projections ----
            key_ps = psB.tile([128, 128], F32, tag="keyps", bufs=1)
            nc.tensor.matmul(key_ps[:], lhsT=Wk0, rhs=csT0, start=True, stop=False)
            nc.tensor.matmul(key_ps[:], lhsT=Wk1, rhs=csT1, start=False, stop=True)
            for b in range(BL):
                nc.vector.tensor_copy(
                    strips[:, b, 31:32], key_ps[:, _pos(b) : _pos(b) + 1]
                )

            proj_ps = psB.tile([128, 6], F32, tag="tiny", bufs=3)
            nc.tensor.matmul(proj_ps[:], lhsT=csT0, rhs=Wc0, start=True, stop=False)
            nc.tensor.matmul(proj_ps[:], lhsT=csT1, rhs=Wc1, start=False, stop=True)
            proj = lp.tile([128, 6], F32)
            nc.vector.tensor_add(proj[:], proj_ps[:], bias6)

            # absorbers: make PE observe the cpk-DMA and DVE-strip ticks once
            absorb = psB.tile([128, 8], F32, tag="tiny", bufs=3, name="absorb")
            nc.tensor.matmul(absorb[:, 0:1], lhsT=indT2, rhs=ones_col,
                             start=True, stop=True, skip_group_check=True)
            nc.tensor.matmul(absorb[0:63, 1:2], lhsT=ones_strip[:],
                             rhs=ones_strip[:, 31:32],
                             start=True, stop=True, skip_group_check=True)

            # |key|^2 -> F_kn2 broadcast to window rows
            kq = lp.tile([128, 128], F32)
            nc.scalar.activation(kq[:], key_ps[:], AF.Square)
            kn2_ps = psB.tile([128, 1], F32, tag="tiny", bufs=3)
            nc.tensor.matmul(kn2_ps[:], lhsT=kq[:], rhs=ones_col, start=True, stop=True)
            kn2 = lp.tile([128, 1], F32)
            nc.vector.tensor_copy(kn2[:], kn2_ps[:])
            fkn_ps = psB.tile([128, 1], F32, tag="tiny", bufs=3)
            nc.tensor.matmul(fkn_ps[:], lhsT=indT2, rhs=kn2[:], start=True, stop=True)
            F_kn2 = lp.tile([128, 1], F32)
            nc.vector.tensor_copy(F_kn2[:], fkn_ps[:])

            # ---- per-batch scalars at rows P(b) ----
            scal = lp.tile([128, 7], F32)
            eb = lp.tile([128, 1], F32)
            nc.scalar.activation(eb[:], proj[:, 0:1], AF.Exp)
            sp_b = lp.tile([128, 1], F32)
            nc.scalar.activation(sp_b[:], eb[:], AF.Ln, bias=1.0)
            nc.vector.tensor_scalar_add(scal[:, 0:1], sp_b[:], 1.0)
            eg = lp.tile([128, 1], F32)
            nc.scalar.activation(eg[:], proj[:, 1:2], AF.Exp, scale=-1.0)
            dg = lp.tile([128, 1], F32)
            nc.vector.tensor_scalar_add(dg[:], eg[:], 1.0)
            gate = lp.tile([128, 1], F32)
            nc.vector.reciprocal(gate[:], dg[:])
            nc.vector.tensor_scalar(
                scal[:, 1:2], gate[:], -1.0, 1.0, op0=OP.mult, op1=OP.add
            )
            e3 = lp.tile([128, 3], F32)
            nc.scalar.activation(e3[:], proj[:, 2:5], AF.Exp)
            ssum = lp.tile([128, 1], F32)
            nc.vector.reduce_sum(ssum[:], e3[:], axis=mybir.AxisListType.X)
            rssum = lp.tile([128, 1], F32)
            nc.vector.reciprocal(rssum[:], ssum[:])
            sh3 = lp.tile([128, 3], F32)
            nc.scalar.mul(sh3[:], e3[:], rssum[:])
            nc.vector.tensor_copy(scal[:, 2:5], sh3[:])
            egm = lp.tile([128, 1], F32)
            nc.scalar.activation(egm[:], proj[:, 5:6], AF.Exp)
            sp_g = lp.tile([128, 1], F32)
            nc.scalar.activation(sp_g[:], egm[:], AF.Ln, bias=1.0)
            nc.vector.tensor_scalar_add(scal[:, 5:6], sp_g[:], 1.0)
            nc.vector.tensor_copy(scal[:, 6:7], gate[:])
            FB_ps = psB.tile([128, 7], F32, tag="tiny", bufs=3)
            nc.tensor.matmul(FB_ps[:], lhsT=indT2, rhs=scal[:], start=True, stop=True)
            FB = lp.tile([128, 7], F32)
            nc.vector.tensor_copy(FB[:], FB_ps[:])
            F_beta = FB[:, 0:1]
            F_g1 = FB[:, 1:2]
            F_s0 = FB[:, 2:3]
            F_s1 = FB[:, 3:4]
            F_s2 = FB[:, 4:5]
            F_gamma = FB[:, 5:6]
            F_gate = FB[:, 6:7]

            t4_all = lp.tile([128, W], F32)
            nc.vector.tensor_scalar_mul(t4_all[:], pw_sb[:], F_g1)

            # ---- light tiles ----
            Lv = lp.tile([128, W], F32)
            y1 = lp.tile([128, W], F32)
            sim = lp.tile([128, W], F32)
            E = lp.tile([128, W], F32)
            G = lp.tile([128, W], F32)
            SH = lp.tile([128, W], F32)
            Lg = lp.tile([128, W], F32)
            P2 = lp.tile([128, W], F32)
            outsb = lp.tile([128, W], F32)
            rs1 = lp.tile([128, 1], F32)
            rs2 = lp.tile([128, 1], F32)
            rS = lp.tile([128, 1], F32)
            gs_all = lp.tile([128, 1], F32)
            S2a = lp.tile([128, 1], F32)
            r2a = lp.tile([128, 1], F32)
            F_gs = lp.tile([128, 1], F32)
            F_r2 = lp.tile([128, 1], F32)
            bl_sb = lp.tile([128, 1], F32)
            br_sb = lp.tile([128, 1], F32)

            dotP = psA.tile([128, W], F32, tag="dotP", bufs=1, name="dotP")
            nrmP = psA.tile([128, W], F32, tag="nrmP", bufs=1, name="nrmP")
            sq_t = {}

            def dots(b):
                j = b // 2
                rows = slice(32 * j, 32 * j + 32)
                st = raw_t[b]
                for t in range(NW):
                    c = NW * (b % 2) + t
                    nc.tensor.matmul(
                        dotP[rows, :],
                        lhsT=strips[:, b, 31 - c : 63 - c],
                        rhs=st[:, t * W : (t + 1) * W],
                        start=(b % 2 == 0) and (t == 0),
                        stop=(b % 2 == 1) and (t == NW - 1),
                        skip_group_check=True,
                        tile_position=(0, 32 * j),
                    )

            def squares(b):
                sq = sqp.tile([128, N], BF16, tag="sq", name=f"sq_{b}")
                st = raw_t[b]
                nc.scalar.activation(sq[:, 0:SA], st[:, 0:SA], AF.Square)
                nc.vector.tensor_mul(sq[:, SA:HALF], st[:, SA:HALF], st[:, SA:HALF])
                nc.vector.tensor_mul(sq[:, HALF:SV], st[:, HALF:SV], st[:, HALF:SV])
                nc.gpsimd.tensor_mul(sq[:, SV:N], st[:, SV:N], st[:, SV:N])
                sq_t[b] = sq

            def norms(b):
                j = b // 2
                rows = slice(32 * j, 32 * j + 32)
                sq = sq_t[b]
                for t in range(NW):
                    c = NW * (b % 2) + t
                    nc.tensor.matmul(
                        nrmP[rows, :],
                        lhsT=ones_strip[:, 31 - c : 63 - c],
                        rhs=sq[:, t * W : (t + 1) * W],
                        start=(b % 2 == 0) and (t == 0),
                        stop=(b % 2 == 1) and (t == NW - 1),
                        skip_group_check=True,
                        tile_position=(0, 32 * j),
                    )

            def stageA(j):
                R = slice(32 * j, 32 * j + 32)
                nc.scalar.activation(Lv[R, :], nrmP[R, :], AF.Ln,
                                     scale=F_kn2[R, :])
                nc.scalar.activation(y1[R, :], Lv[R, :], AF.Exp, scale=-0.5)
                nc.vector.tensor_mul(sim[R, :], dotP[R, :], y1[R, :])
                nc.scalar.activation(E[R, :], sim[R, :], AF.Exp,
                                     scale=F_beta[R, :], accum_out=rs1[R, :])

            for it in range(BL + 1):
                if it < BL:
                    dots(it)
                    squares(it)
                if it >= 1:
                    norms(it - 1)
                    if (it - 1) % 2 == 1:
                        stageA((it - 1) // 2)

            # ---- tail: batch-sum-broadcast via block-diag ind_bb in ONE
            # matmul, then full-width chain ----
            sps = psB.tile([128, 1], F32, tag="tiny", bufs=3, name="sps")
            nc.tensor.matmul(sps[:], lhsT=ind_bb, rhs=rs1[:],
                             start=True, stop=True, skip_group_check=True)
            nc.vector.reciprocal(rS[:], sps[:])
            nc.vector.tensor_mul(F_gs[:], F_gate, rS[:])
            nc.vector.scalar_tensor_tensor(
                G[:], E[:], F_gs[:], t4_all[:], op0=OP.mult, op1=OP.add
            )
            blps = psB.tile([128, 1], F32, tag="tiny", bufs=3, name="blps")
            nc.tensor.matmul(blps[:], lhsT=pnext, rhs=G[:, 0:1],
                             start=True, stop=True, skip_group_check=True)
            brps = psB.tile([128, 1], F32, tag="tiny", bufs=3, name="brps")
            nc.tensor.matmul(brps[:], lhsT=pprev, rhs=G[:, W - 1 : W],
                             start=True, stop=True, skip_group_check=True)
            nc.vector.tensor_copy(bl_sb[:], blps[:])
            nc.vector.tensor_copy(br_sb[:], brps[:])
            nc.scalar.mul(SH[:], G[:], F_s1)
            nc.vector.scalar_tensor_tensor(
                SH[:, 0 : W - 1], G[:, 1:W], F_s0, SH[:, 0 : W - 1],
                op0=OP.mult, op1=OP.add,
            )
            nc.vector.scalar_tensor_tensor(
                SH[:, 1:W], G[:, 0 : W - 1], F_s2, SH[:, 1:W],
                op0=OP.mult, op1=OP.add,
            )
            nc.vector.scalar_tensor_tensor(
                SH[:, W - 1 : W], bl_sb[:], F_s0, SH[:, W - 1 : W],
                op0=OP.mult, op1=OP.add,
            )
            nc.vector.scalar_tensor_tensor(
                SH[:, 0:1], br_sb[:], F_s2, SH[:, 0:1],
                op0=OP.mult, op1=OP.add,
            )
            nc.scalar.activation(Lg[:], SH[:], AF.Ln,
                                 bias=cpk_raw[:, CP_EPS : CP_EPS + 1])
            nc.scalar.activation(P2[:], Lg[:], AF.Exp,
                                 scale=F_gamma, accum_out=rs2[:])
            s2ps = psB.tile([128, 1], F32, tag="tiny", bufs=3, name="s2ps")
            nc.tensor.matmul(s2ps[:], lhsT=ind_bb, rhs=rs2[:],
                             start=True, stop=True, skip_group_check=True)
            nc.vector.tensor_scalar_add(S2a[:], s2ps[:], EPS)
            nc.vector.reciprocal(F_r2[:], S2a[:])
            nc.scalar.mul(outsb[:], P2[:], F_r2)
            nc.sync.dma_start(
                out_d[:].rearrange("b (q f) -> (b q) f", f=W), outsb[:]
            )
    nc.compile()
    return nc


def _get_nc():
    global _NC
    if _NC is None:
        _NC = build_nc()
    return _NC


def _enable_profiling():
    """Install the axon NTFF profile hook; the agent image lacks
    antenv.axon_hooks, so shim it and register the ctypes-based hook."""
    import types

    import concourse.bass_utils as bu

    bu.upload_artifacts = lambda tmpdir: tmpdir
    try:
        from antenv.axon_hooks import get_axon_ntff_profile_hook  # noqa: F401

        return
    except ImportError:
        pass
    import antenv

    mod = types.ModuleType("antenv.axon_hooks")
    _holder = {}
    mod.set_axon_ntff_profile_hook = lambda h: _holder.__setitem__("h", h)
    mod.get_axon_ntff_profile_hook = lambda: _holder.get("h")
    sys.modules["antenv.axon_hooks"] = mod
    antenv.axon_hooks = mod
    from trn_agent_boot.trn_boot import _ntff_profile_via_ctypes

    mod.set_axon_ntff_profile_hook(
        _ntff_profile_via_ctypes("/opt/axon/libaxon_pjrt.so")
    )


def kernel(**inputs):
    global LAST_RESULTS
    mem = np.ascontiguousarray(np.asarray(inputs["memory"], dtype=np.float32))
    cs = np.ascontiguousarray(np.asarray(inputs["controller_state"], dtype=np.float32))
    pw = np.ascontiguousarray(np.asarray(inputs["previous_weights"], dtype=np.float32))
    Wk = np.ascontiguousarray(np.asarray(inputs["Wk"], dtype=np.float32))
    Wcat = np.concatenate(
        [
            np.asarray(inputs["Wb"], np.float32),
            np.asarray(inputs["Wg"], np.float32),
            np.asarray(inputs["Ws"], np.float32),
            np.asarray(inputs["Wgam"], np.float32),
        ],
        axis=1,
    )
    brow = np.concatenate(
        [
            np.asarray(inputs["bb"], np.float32),
            np.asarray(inputs["bg"], np.float32),
            np.asarray(inputs["bs"], np.float32),
            np.asarray(inputs["bgam"], np.float32),
        ]
    )
    pos = np.array([_pos(b) for b in range(BL)])

    # shard: core c gets batches [c*BL, (c+1)*BL); memory pre-transposed
    memT = np.ascontiguousarray(
        mem.reshape(NCORES, BL, N, D).transpose(0, 1, 3, 2)
    )
    import ml_dtypes
    memT = memT.astype(ml_dtypes.bfloat16)
    csT_all = cs.reshape(NCORES, BL, C).transpose(0, 2, 1)  # (8, C, BL)
    pw_sh = pw.reshape(NCORES, BL, N)

    # packed external input: csT0|csT1|Wk0|Wk1|Wc0|Wc1|bias6
    ext = np.zeros((NCORES, 128, EXTW), np.float32)
    ext[:, :, EXT_WK0 : EXT_WK0 + 128] = Wk[0:128, :][None]
    ext[:, :, EXT_WK1 : EXT_WK1 + 128] = Wk[128:256, :][None]
    ext[:, :, EXT_WC0 : EXT_WC0 + 6] = Wcat[0:128, :][None]
    ext[:, :, EXT_WC1 : EXT_WC1 + 6] = Wcat[128:256, :][None]
    ext[:, pos, EXT_B6 : EXT_B6 + 6] = brow[None, None, :]
    for bi in range(BL):
        ext[:, :, EXT_CS0 + pos[bi]] = csT_all[:, 0:128, bi]
        ext[:, :, EXT_CS1 + pos[bi]] = csT_all[:, 128:256, bi]

    in_maps = [
        {
            "memT": memT[c],
            "ext": np.ascontiguousarray(ext[c]),
            "pw": np.ascontiguousarray(pw_sh[c]),
        }
        for c in range(NCORES)
    ]
    nc = _get_nc()
    if PROFILE:
        _enable_profiling()
    res = run_bass_kernel_spmd(nc, in_maps, list(range(NCORES)), trace=PROFILE)
    LAST_RESULTS = res
    out = np.concatenate([r["out"] for r in res.results], axis=0)
    return out.astype(np.float32)


# revision 15
# speedup vs baseline: 1.4053x; 1.0081x over previous
"""Trainium2 Bass kernel for a differentiable addressing head (NTM-style).

Computes, for each batch b:
    key   = cs @ Wk;  beta = softplus(cs@Wb+bb)+1;  gate = sigmoid(cs@Wg+bg)
    shift = softmax(cs@Ws+bs);  gamma = softplus(cs@Wgam+bgam)+1
    sim   = (key . mem[n]) / (|key||mem[n]| + eps)
    cw    = softmax(beta * sim);  g = gate*cw + (1-gate)*pw
    sh    = circular_conv(g, shift);  w = (sh+1e-8)^gamma / (sum + eps)

Sharding: data-parallel over batch across 8 cores (8 batches/core).

Heavy pipeline per batch b (stripe = full row [128(D), 8192(N)] bf16 with
16 KB-contiguous DMA descriptors per partition, fetched in halves):
  DMA(b) -> dots(b) on PE -> squares(b) OUT-OF-PLACE (ACT/DVE/GPSIMD
  split) -> norms(b) on PE two batches behind dots.

The light phase for each 2-batch group is cut into 6 stages (A..F) that
are pipelined across heavy-loop iterations: each stage's PE micro-matmul
only depends on ACT/DVE work from >= 1 iteration earlier, so the
in-order PE never stalls on the light chain. Per-batch scalars live at
partitions P(b)=32*(b//2)+(b%2) so every tiny fp32 matmul has 32-aligned
tile positions.

Light layout: row p = 16b + t holds n in [512t, 512(t+1)) of batch b.

Self-contained: hardcodes shapes B=64, N=8192, D=128, C=256.
"""

import os
import sys

import numpy as np

for _p in ("/opt/trn_rl_repo", "/opt/pypackages"):
    if _p not in sys.path and os.path.isdir(_p):
        sys.path.insert(0, _p)

import concourse.bacc as bacc
import concourse.bass as bass
import concourse.tile as tile
from concourse import mybir
from concourse.bass_utils import run_bass_kernel_spmd

F32 = mybir.dt.float32
BF16 = mybir.dt.bfloat16
AF = mybir.ActivationFunctionType
OP = mybir.AluOpType

B, N, D, C = 64, 8192, 128, 256
NCORES = 8
BL = B // NCORES          # batches per core = 8
NW = 16                   # 512-wide windows per batch row
W = N // NW               # 512, window width (= light-tile free dim)
EPS = 1e-8
NG = BL // 2              # 2-batch light groups

# square-engine split (multiples of W): [0:SA]=ACT, [SA:SV]=DVE, [SV:N]=GP
SA = 3072
SV = 7168
HALF = N // 2

# packed external-input layout (columns of ext[128, EXTW]):
EXT_CS0, EXT_CS1 = 0, 128
EXT_WK0, EXT_WK1 = 256, 384
EXT_WC0, EXT_WC1 = 512, 518
EXT_B6 = 524
EXTW = 530
EXT_STAGED = 524          # cols staged for fp32 matmuls (csT/Wk/Wc)

# packed const layout (columns of cpk[128, CPKW]):
CP_IND2 = 0
CP_INDT2 = 2
CP_PN = 130
CP_PP = 258
CP_IP = 386
CP_BB = 514
CP_ONE = 642
CP_STRIP = 643
CP_EPS = 706
CPKW = 707
CP_STAGED = 643           # ind2..ones_col staged f32 (strip staged bf16)

_NC = None
PROFILE = False
LAST_RESULTS = None


def _pos(b):
    return 32 * (b // 2) + (b % 2)


def _consts():
    cpk = np.zeros((128, CPKW), np.float32)
    for p in range(128):
        cpk[p, CP_IND2 + (p % 32) // 16] = 1.0
    for j in range(4):
        for a in range(2):
            for t in range(16):
                cpk[32 * j + a, CP_INDT2 + 32 * j + 16 * a + t] = 1.0
    for m in range(128):
        bb, t = m // 16, m % 16
        cpk[16 * bb + (t + 1) % 16, CP_PN + m] = 1.0
        cpk[16 * bb + (t - 1) % 16, CP_PP + m] = 1.0
    for j in range(4):
        for a in range(2):
            for t in range(16):
                cpk[32 * j + 16 * a + t, CP_IP + 32 * j + a] = 1.0
    for p in range(128):
        for m in range(128):
            if p // 16 == m // 16:
                cpk[p, CP_BB + m] = 1.0
    cpk[:, CP_ONE] = 1.0
    cpk[:, CP_STRIP + 31] = 1.0
    cpk[:, CP_EPS] = EPS
    return cpk


def _patch_act_tables():
    """Keep exp+ln+square+copy in ONE ACT table set (a swap costs 1.3us)."""
    import concourse.hw_specs as hw_specs  # noqa: F401

    if getattr(bacc, "_act_tables_patched", False):
        return
    orig = bacc.get_activation_tables

    def filtered(module_arch):
        t = orig(module_arch)
        pref = "natural_log_exp_and_others"
        if pref in t:
            mine = {
                AF.Exp, AF.Ln, AF.Square, AF.Copy, AF.Identity, AF.MemsetZero
            } & t[pref]
            for k in t:
                if k != pref:
                    t[k] = t[k] - mine
        return t

    bacc.get_activation_tables = filtered
    bacc._act_tables_patched = True


def build_nc():
    _patch_act_tables()
    nc = bacc.Bacc()

    memT_d = nc.dram_tensor("memT", [BL, D, N], BF16, kind="ExternalInput")
    ext_d = nc.dram_tensor("ext", [128, EXTW], F32, kind="ExternalInput")
    pw_d = nc.dram_tensor("pw", [BL, N], F32, kind="ExternalInput")
    out_d = nc.dram_tensor("out", [BL, N], F32, kind="ExternalOutput")
    cpk_c = nc.inline_tensor(_consts(), "cpk_c")

    with tile.TileContext(nc) as tc:
        with (
            tc.tile_pool(name="const", bufs=1) as cp,
            tc.tile_pool(name="mem", bufs=6) as memp,
            tc.tile_pool(name="sq", bufs=3) as sqp,
            tc.tile_pool(name="light", bufs=1) as lp,
            tc.tile_pool(name="psmm", bufs=2, space="PSUM") as psA,
            tc.tile_pool(name="pstiny", bufs=2, space="PSUM") as psB,
        ):
            # strips memset depends on nothing: issue before everything
            strips = cp.tile([128, BL, 63], BF16)
            nc.vector.memset(strips[:], 0.0)

            # ---- packed input/const DMAs + first stripe quarters ----
            ext_raw = cp.tile([128, EXTW], F32, name="ext_raw")
            nc.scalar.dma_start(ext_raw[:], ext_d[:])
            memT_ap = memT_d[:]
            raw_t = []
            st0 = memp.tile([128, N], BF16, tag="raw", name="raw_0")
            raw_t.append(st0)
            Q = N // 4
            for q in range(4):
                nc.sync.dma_start(st0[:, q * Q : (q + 1) * Q],
                                  memT_ap[0][:, q * Q : (q + 1) * Q])
            st1 = memp.tile([128, N], BF16, tag="raw", name="raw_1")
            raw_t.append(st1)
            nc.sync.dma_start(st1[:, 0:HALF], memT_ap[1][:, 0:HALF])
            cpk_raw = cp.tile([128, CPKW], F32, name="cpk_raw")
            nc.scalar.dma_start(cpk_raw[:], cpk_c[:])
            nc.sync.dma_start(st1[:, HALF:N], memT_ap[1][:, HALF:N])
            pw_sb = cp.tile([128, W], F32)
            nc.scalar.dma_start(pw_sb[:], pw_d[:].rearrange("b (q f) -> (b q) f", f=W))
            for b in range(2, BL):
                st = memp.tile([128, N], BF16, tag="raw", name=f"raw_{b}")
                eng = nc.sync
                eng.dma_start(st[:, 0:HALF], memT_ap[b][:, 0:HALF])
                eng.dma_start(st[:, HALF:N], memT_ap[b][:, HALF:N])
                raw_t.append(st)

            # packed tiles are each written by ONE DMA instruction, so a
            # matmul reading any slices of one pack carries one DMA tick;
            # absorbers below make PE observe each pack's tick once.
            ones_strip = cp.tile([128, 63], BF16, name="strip_g")
            nc.vector.tensor_copy(ones_strip[:], cpk_raw[:, CP_STRIP : CP_STRIP + 63])

            csT0 = ext_raw[:, EXT_CS0 : EXT_CS0 + 128]
            csT1 = ext_raw[:, EXT_CS1 : EXT_CS1 + 128]
            Wk0 = ext_raw[:, EXT_WK0 : EXT_WK0 + 128]
            Wk1 = ext_raw[:, EXT_WK1 : EXT_WK1 + 128]
            Wc0 = ext_raw[:, EXT_WC0 : EXT_WC0 + 6]
            Wc1 = ext_raw[:, EXT_WC1 : EXT_WC1 + 6]
            bias6 = ext_raw[:, EXT_B6 : EXT_B6 + 6]
            ind2 = cpk_raw[:, CP_IND2 : CP_IND2 + 2]
            indT2 = cpk_raw[:, CP_INDT2 : CP_INDT2 + 128]
            pnext = cpk_raw[:, CP_PN : CP_PN + 128]
            pprev = cpk_raw[:, CP_PP : CP_PP + 128]
            indP = cpk_raw[:, CP_IP : CP_IP + 128]
            ind_bb = cpk_raw[:, CP_BB : CP_BB + 128]
            ones_col = cpk_raw[:, CP_ONE : CP_ONE + 1]

            # ---- projections ----
            key_ps = psB.tile([128, 128], F32, tag="keyps", bufs=1)
            nc.tensor.matmul(key_ps[:], lhsT=Wk0, rhs=csT0, start=True, stop=False)
            nc.tensor.matmul(key_ps[:], lhsT=Wk1, rhs=csT1, start=False, stop=True)
            for b in range(BL):
                nc.vector.tensor_copy(
                    strips[:, b, 31:32], key_ps[:, _pos(b) : _pos(b) + 1]
                )

            proj_ps = psB.tile([128, 6], F32, tag="tiny", bufs=3)
            nc.tensor.matmul(proj_ps[:], lhsT=csT0, rhs=Wc0, start=True, stop=False)
            nc.tensor.matmul(proj_ps[:], lhsT=csT1, rhs=Wc1, start=False, stop=True)
            proj = lp.tile([128, 6], F32)
            nc.vector.tensor_add(proj[:], proj_ps[:], bias6)

            # absorbers: make PE observe the cpk-DMA and DVE-strip ticks once
            absorb = psB.tile([128, 8], F32, tag="tiny", bufs=3, name="absorb")
            nc.tensor.matmul(absorb[:, 0:1], lhsT=indT2, rhs=ones_col,
                             start=True, stop=True, skip_group_check=True)
            nc.tensor.matmul(absorb[0:63, 1:2], lhsT=ones_strip[:],
                             rhs=ones_strip[:, 31:32],
                             start=True, stop=True, skip_group_check=True)

            # |key|^2 -> F_kn2 broadcast to window rows
            kq = lp.tile([128, 128], F32)
            nc.scalar.activation(kq[:], key_ps[:], AF.Square)
            kn2_ps = psB.tile([128, 1], F32, tag="tiny", bufs=3)
            nc.tensor.matmul(kn2_ps[:], lhsT=kq[:], rhs=ones_col, start=True, stop=True)
            kn2 = lp.tile([128, 1], F32)
            nc.vector.tensor_copy(kn2[:], kn2_ps[:])
            fkn_ps = psB.tile([128, 1], F32, tag="tiny", bufs=3)
            nc.tensor.matmul(fkn_ps[:], lhsT=indT2, rhs=kn2[:], start=True, stop=True)
            F_kn2 = lp.tile([128, 1], F32)
            nc.vector.tensor_copy(F_kn2[:], fkn_ps[:])

            # ---- per-batch scalars at rows P(b) ----
            scal = lp.tile([128, 7], F32)
            eb = lp.tile([128, 1], F32)
            nc.scalar.activation(eb[:], proj[:, 0:1], AF.Exp)
            sp_b = lp.tile([128, 1], F32)
            nc.scalar.activation(sp_b[:], eb[:], AF.Ln, bias=1.0)
            nc.vector.tensor_scalar_add(scal[:, 0:1], sp_b[:], 1.0)
            eg = lp.tile([128, 1], F32)
            nc.scalar.activation(eg[:], proj[:, 1:2], AF.Exp, scale=-1.0)
            dg = lp.tile([128, 1], F32)
            nc.vector.tensor_scalar_add(dg[:], eg[:], 1.0)
            gate = lp.tile([128, 1], F32)
            nc.vector.reciprocal(gate[:], dg[:])
            nc.vector.tensor_scalar(
                scal[:, 1:2], gate[:], -1.0, 1.0, op0=OP.mult, op1=OP.add
            )
            e3 = lp.tile([128, 3], F32)
            nc.scalar.activation(e3[:], proj[:, 2:5], AF.Exp)
            ssum = lp.tile([128, 1], F32)
            nc.vector.reduce_sum(ssum[:], e3[:], axis=mybir.AxisListType.X)
            rssum = lp.tile([128, 1], F32)
            nc.vector.reciprocal(rssum[:], ssum[:])
            sh3 = lp.tile([128, 3], F32)
            nc.scalar.mul(sh3[:], e3[:], rssum[:])
            nc.vector.tensor_copy(scal[:, 2:5], sh3[:])
            egm = lp.tile([128, 1], F32)
            nc.scalar.activation(egm[:], proj[:, 5:6], AF.Exp)
            sp_g = lp.tile([128, 1], F32)
            nc.scalar.activation(sp_g[:], egm[:], AF.Ln, bias=1.0)
            nc.vector.tensor_scalar_add(scal[:, 5:6], sp_g[:], 1.0)
            nc.vector.tensor_copy(scal[:, 6:7], gate[:])
            FB_ps = psB.tile([128, 7], F32, tag="tiny", bufs=3)
            nc.tensor.matmul(FB_ps[:], lhsT=indT2, rhs=scal[:], start=True, stop=True)
            FB = lp.tile([128, 7], F32)
            nc.vector.tensor_copy(FB[:], FB_ps[:])
            F_beta = FB[:, 0:1]
            F_g1 = FB[:, 1:2]
            F_s0 = FB[:, 2:3]
            F_s1 = FB[:, 3:4]
            F_s2 = FB[:, 4:5]
            F_gamma = FB[:, 5:6]
            F_gate = FB[:, 6:7]

            t4_all = lp.tile([128, W], F32)
            nc.vector.tensor_scalar_mul(t4_all[:], pw_sb[:], F_g1)

            # ---- light tiles ----
            Lv = lp.tile([128, W], F32)
            y1 = lp.tile([128, W], F32)
            sim = lp.tile([128, W], F32)
            E = lp.tile([128, W], F32)
            G = lp.tile([128, W], F32)
            SH = lp.tile([128, W], F32)
            Lg = lp.tile([128, W], F32)
            P2 = lp.tile([128, W], F32)
            outsb = lp.tile([128, W], F32)
            rs1 = lp.tile([128, 1], F32)
            rs2 = lp.tile([128, 1], F32)
            rS = lp.tile([128, 1], F32)
            gs_all = lp.tile([128, 1], F32)
            S2a = lp.tile([128, 1], F32)
            r2a = lp.tile([128, 1], F32)
            F_gs = lp.tile([128, 1], F32)
            F_r2 = lp.tile([128, 1], F32)
            bl_sb = lp.tile([128, 1], F32)
            br_sb = lp.tile([128, 1], F32)

            dotP = psA.tile([128, W], F32, tag="dotP", bufs=1, name="dotP")
            nrmP = psA.tile([128, W], F32, tag="nrmP", bufs=1, name="nrmP")
            sq_t = {}

            def dots(b):
                j = b // 2
                rows = slice(32 * j, 32 * j + 32)
                st = raw_t[b]
                for t in range(NW):
                    c = NW * (b % 2) + t
                    nc.tensor.matmul(
                        dotP[rows, :],
                        lhsT=strips[:, b, 31 - c : 63 - c],
                        rhs=st[:, t * W : (t + 1) * W],
                        start=(b % 2 == 0) and (t == 0),
                        stop=(b % 2 == 1) and (t == NW - 1),
                        skip_group_check=True,
                        tile_position=(0, 32 * j),
                    )

            def squares(b):
                sq = sqp.tile([128, N], BF16, tag="sq", name=f"sq_{b}")
                st = raw_t[b]
                nc.scalar.activation(sq[:, 0:SA], st[:, 0:SA], AF.Square)
                nc.vector.tensor_mul(sq[:, SA:HALF], st[:, SA:HALF], st[:, SA:HALF])
                nc.vector.tensor_mul(sq[:, HALF:SV], st[:, HALF:SV], st[:, HALF:SV])
                nc.gpsimd.tensor_mul(sq[:, SV:N], st[:, SV:N], st[:, SV:N])
                sq_t[b] = sq

            def norms(b):
                j = b // 2
                rows = slice(32 * j, 32 * j + 32)
                sq = sq_t[b]
                for t in range(NW):
                    c = NW * (b % 2) + t
                    nc.tensor.matmul(
                        nrmP[rows, :],
                        lhsT=ones_strip[:, 31 - c : 63 - c],
                        rhs=sq[:, t * W : (t + 1) * W],
                        start=(b % 2 == 0) and (t == 0),
                        stop=(b % 2 == 1) and (t == NW - 1),
                        skip_group_check=True,
                        tile_position=(0, 32 * j),
                    )

            def stageA(j):
                R = slice(32 * j, 32 * j + 32)
                nc.scalar.activation(Lv[R, :], nrmP[R, :], AF.Ln,
                                     scale=F_kn2[R, :])
                nc.scalar.activation(y1[R, :], Lv[R, :], AF.Exp, scale=-0.5)
                nc.vector.tensor_mul(sim[R, :], dotP[R, :], y1[R, :])
                nc.scalar.activation(E[R, :], sim[R, :], AF.Exp,
                                     scale=F_beta[R, :], accum_out=rs1[R, :])

            for it in range(BL + 2):
                if it < BL:
                    dots(it)
                    squares(it)
                if it >= 2:
                    norms(it - 2)
                    if (it - 2) % 2 == 1:
                        stageA((it - 2) // 2)

            # ---- tail: batch-sum-broadcast via block-diag ind_bb in ONE
            # matmul, then full-width chain ----
            sps = psB.tile([128, 1], F32, tag="tiny", bufs=3, name="sps")
            nc.tensor.matmul(sps[:], lhsT=ind_bb, rhs=rs1[:],
                             start=True, stop=True, skip_group_check=True)
            nc.vector.reciprocal(rS[:], sps[:])
            nc.vector.tensor_mul(F_gs[:], F_gate, rS[:])
            nc.vector.scalar_tensor_tensor(
                G[:], E[:], F_gs[:], t4_all[:], op0=OP.mult, op1=OP.add
            )
            blps = psB.tile([128, 1], F32, tag="tiny", bufs=3, name="blps")
            nc.tensor.matmul(blps[:], lhsT=pnext, rhs=G[:, 0:1],
                             start=True, stop=True, skip_group_check=True)
            brps = psB.tile([128, 1], F32, tag="tiny", bufs=3, name="brps")
            nc.tensor.matmul(brps[:], lhsT=pprev, rhs=G[:, W - 1 : W],
                             start=True, stop=True, skip_group_check=True)
            nc.vector.tensor_copy(bl_sb[:], blps[:])
            nc.vector.tensor_copy(br_sb[:], brps[:])
            nc.scalar.mul(SH[:], G[:], F_s1)
            nc.vector.scalar_tensor_tensor(
                SH[:, 0 : W - 1], G[:, 1:W], F_s0, SH[:, 0 : W - 1],
                op0=OP.mult, op1=OP.add,
            )
            nc.vector.scalar_tensor_tensor(
                SH[:, 1:W], G[:, 0 : W - 1], F_s2, SH[:, 1:W],
                op0=OP.mult, op1=OP.add,
            )
            nc.vector.scalar_tensor_tensor(
                SH[:, W - 1 : W], bl_sb[:], F_s0, SH[:, W - 1 : W],
                op0=OP.mult, op1=OP.add,
            )
            nc.vector.scalar_tensor_tensor(
                SH[:, 0:1], br_sb[:], F_s2, SH[:, 0:1],
                op0=OP.mult, op1=OP.add,
            )
            nc.scalar.activation(Lg[:], SH[:], AF.Ln,
                                 bias=cpk_raw[:, CP_EPS : CP_EPS + 1])
            nc.scalar.activation(P2[:], Lg[:], AF.Exp,
                                 scale=F_gamma, accum_out=rs2[:])
            s2ps = psB.tile([128, 1], F32, tag="tiny", bufs=3, name="s2ps")
            nc.tensor.matmul(s2ps[:], lhsT=ind_bb, rhs=rs2[:],
                             start=True, stop=True, skip_group_check=True)
            nc.vector.tensor_scalar_add(S2a[:], s2ps[:], EPS)
            nc.vector.reciprocal(F_r2[:], S2a[:])
            nc.scalar.mul(outsb[:], P2[:], F_r2)
            nc.sync.dma_start(
                out_d[:].rearrange("b (q f) -> (b q) f", f=W), outsb[:]
            )
    nc.compile()
    return nc


def _get_nc():
    global _NC
    if _NC is None:
        _NC = build_nc()
    return _NC


def _enable_profiling():
    """Install the axon NTFF profile hook; the agent image lacks
    antenv.axon_hooks, so shim it and register the ctypes-based hook."""
    import types

    import concourse.bass_utils as bu

    bu.upload_artifacts = lambda tmpdir: tmpdir
    try:
        from antenv.axon_hooks import get_axon_ntff_profile_hook  # noqa: F401

        return
    except ImportError:
        pass
    import antenv

    mod = types.ModuleType("antenv.axon_hooks")
    _holder = {}
    mod.set_axon_ntff_profile_hook = lambda h: _holder.__setitem__("h", h)
    mod.get_axon_ntff_profile_hook = lambda: _holder.get("h")
    sys.modules["antenv.axon_hooks"] = mod
    antenv.axon_hooks = mod
    from trn_agent_boot.trn_boot import _ntff_profile_via_ctypes

    mod.set_axon_ntff_profile_hook(
        _ntff_profile_via_ctypes("/opt/axon/libaxon_pjrt.so")
    )


def kernel(**inputs):
    global LAST_RESULTS
    mem = np.ascontiguousarray(np.asarray(inputs["memory"], dtype=np.float32))
    cs = np.ascontiguousarray(np.asarray(inputs["controller_state"], dtype=np.float32))
    pw = np.ascontiguousarray(np.asarray(inputs["previous_weights"], dtype=np.float32))
    Wk = np.ascontiguousarray(np.asarray(inputs["Wk"], dtype=np.float32))
    Wcat = np.concatenate(
        [
            np.asarray(inputs["Wb"], np.float32),
            np.asarray(inputs["Wg"], np.float32),
            np.asarray(inputs["Ws"], np.float32),
            np.asarray(inputs["Wgam"], np.float32),
        ],
        axis=1,
    )
    brow = np.concatenate(
        [
            np.asarray(inputs["bb"], np.float32),
            np.asarray(inputs["bg"], np.float32),
            np.asarray(inputs["bs"], np.float32),
            np.asarray(inputs["bgam"], np.float32),
        ]
    )
    pos = np.array([_pos(b) for b in range(BL)])

    # shard: core c gets batches [c*BL, (c+1)*BL); memory pre-transposed
    memT = np.ascontiguousarray(
        mem.reshape(NCORES, BL, N, D).transpose(0, 1, 3, 2)
    )
    import ml_dtypes
    memT = memT.astype(ml_dtypes.bfloat16)
    csT_all = cs.reshape(NCORES, BL, C).transpose(0, 2, 1)  # (8, C, BL)
    pw_sh = pw.reshape(NCORES, BL, N)

    # packed external input: csT0|csT1|Wk0|Wk1|Wc0|Wc1|bias6
    ext = np.zeros((NCORES, 128, EXTW), np.float32)
    ext[:, :, EXT_WK0 : EXT_WK0 + 128] = Wk[0:128, :][None]
    ext[:, :, EXT_WK1 : EXT_WK1 + 128] = Wk[128:256, :][None]
    ext[:, :, EXT_WC0 : EXT_WC0 + 6] = Wcat[0:128, :][None]
    ext[:, :, EXT_WC1 : EXT_WC1 + 6] = Wcat[128:256, :][None]
    ext[:, pos, EXT_B6 : EXT_B6 + 6] = brow[None, None, :]
    for bi in range(BL):
        ext[:, :, EXT_CS0 + pos[bi]] = csT_all[:, 0:128, bi]
        ext[:, :, EXT_CS1 + pos[bi]] = csT_all[:, 128:256, bi]

    in_maps = [
        {
            "memT": memT[c],
            "ext": np.ascontiguousarray(ext[c]),
            "pw": np.ascontiguousarray(pw_sh[c]),
        }
        for c in range(NCORES)
    ]
    nc = _get_nc()
    if PROFILE:
        _enable_profiling()
    res = run_bass_kernel_spmd(nc, in_maps, list(range(NCORES)), trace=PROFILE)
    LAST_RESULTS = res
    out = np.concatenate([r["out"] for r in res.results], axis=0)
    return out.astype(np.float32)


# revision 16
# speedup vs baseline: 1.5123x; 1.0761x over previous
"""Trainium2 Bass kernel for a differentiable addressing head (NTM-style).

Computes, for each batch b:
    key   = cs @ Wk;  beta = softplus(cs@Wb+bb)+1;  gate = sigmoid(cs@Wg+bg)
    shift = softmax(cs@Ws+bs);  gamma = softplus(cs@Wgam+bgam)+1
    sim   = (key . mem[n]) / (|key||mem[n]| + eps)
    cw    = softmax(beta * sim);  g = gate*cw + (1-gate)*pw
    sh    = circular_conv(g, shift);  w = (sh+1e-8)^gamma / (sum + eps)

Sharding: data-parallel over batch across 8 cores (8 batches/core).

Heavy pipeline per batch b (stripe = full row [128(D), 8192(N)] bf16 with
16 KB-contiguous DMA descriptors per partition, fetched in halves):
  DMA(b) -> dots(b) on PE -> squares(b) OUT-OF-PLACE (ACT/DVE/GPSIMD
  split) -> norms(b) on PE two batches behind dots.

The light phase for each 2-batch group is cut into 6 stages (A..F) that
are pipelined across heavy-loop iterations: each stage's PE micro-matmul
only depends on ACT/DVE work from >= 1 iteration earlier, so the
in-order PE never stalls on the light chain. Per-batch scalars live at
partitions P(b)=32*(b//2)+(b%2) so every tiny fp32 matmul has 32-aligned
tile positions.

Light layout: row p = 16b + t holds n in [512t, 512(t+1)) of batch b.

Self-contained: hardcodes shapes B=64, N=8192, D=128, C=256.
"""

import os
import sys

import numpy as np

for _p in ("/opt/trn_rl_repo", "/opt/pypackages"):
    if _p not in sys.path and os.path.isdir(_p):
        sys.path.insert(0, _p)

import concourse.bacc as bacc
import concourse.bass as bass
import concourse.tile as tile
from concourse import mybir
from concourse.bass_utils import run_bass_kernel_spmd

F32 = mybir.dt.float32
BF16 = mybir.dt.bfloat16
AF = mybir.ActivationFunctionType
OP = mybir.AluOpType

B, N, D, C = 64, 8192, 128, 256
NCORES = 8
BL = B // NCORES          # batches per core = 8
NW = 16                   # 512-wide windows per batch row
W = N // NW               # 512, window width (= light-tile free dim)
EPS = 1e-8
NG = BL // 2              # 2-batch light groups

# square-engine split (multiples of W): [0:SA]=ACT, [SA:SV]=DVE, [SV:N]=GP
SA = 3072
SV = 7168
HALF = N // 2

# packed external-input layout (columns of ext[128, EXTW]):
EXT_CS0, EXT_CS1 = 0, 128
EXT_WK0, EXT_WK1 = 256, 384
EXT_WC0, EXT_WC1 = 512, 518
EXT_B6 = 524
EXTW = 530
EXT_STAGED = 524          # cols staged for fp32 matmuls (csT/Wk/Wc)

# packed const layout (columns of cpk[128, CPKW]):
CP_IND2 = 0
CP_INDT2 = 2
CP_PN = 130
CP_PP = 258
CP_IP = 386
CP_BB = 514
CP_ONE = 642
CP_STRIP = 643
CP_EPS = 706
CPKW = 707
CP_STAGED = 643           # ind2..ones_col staged f32 (strip staged bf16)

_NC = None
PROFILE = False
LAST_RESULTS = None


def _pos(b):
    return 32 * (b // 2) + (b % 2)


def _consts():
    cpk = np.zeros((128, CPKW), np.float32)
    for p in range(128):
        cpk[p, CP_IND2 + (p % 32) // 16] = 1.0
    for j in range(4):
        for a in range(2):
            for t in range(16):
                cpk[32 * j + a, CP_INDT2 + 32 * j + 16 * a + t] = 1.0
    for m in range(128):
        bb, t = m // 16, m % 16
        cpk[16 * bb + (t + 1) % 16, CP_PN + m] = 1.0
        cpk[16 * bb + (t - 1) % 16, CP_PP + m] = 1.0
    for j in range(4):
        for a in range(2):
            for t in range(16):
                cpk[32 * j + 16 * a + t, CP_IP + 32 * j + a] = 1.0
    for p in range(128):
        for m in range(128):
            if p // 16 == m // 16:
                cpk[p, CP_BB + m] = 1.0
    cpk[:, CP_ONE] = 1.0
    cpk[:, CP_STRIP + 31] = 1.0
    cpk[:, CP_EPS] = EPS
    return cpk


def _patch_act_tables():
    """Keep exp+ln+square+copy in ONE ACT table set (a swap costs 1.3us)."""
    import concourse.hw_specs as hw_specs  # noqa: F401

    if getattr(bacc, "_act_tables_patched", False):
        return
    orig = bacc.get_activation_tables

    def filtered(module_arch):
        t = orig(module_arch)
        pref = "natural_log_exp_and_others"
        if pref in t:
            mine = {
                AF.Exp, AF.Ln, AF.Square, AF.Copy, AF.Identity, AF.MemsetZero
            } & t[pref]
            for k in t:
                if k != pref:
                    t[k] = t[k] - mine
        return t

    bacc.get_activation_tables = filtered
    bacc._act_tables_patched = True


def build_nc():
    _patch_act_tables()
    nc = bacc.Bacc()

    memT_d = nc.dram_tensor("memT", [BL, D, N], BF16, kind="ExternalInput")
    ext_d = nc.dram_tensor("ext", [128, EXTW], F32, kind="ExternalInput")
    pw_d = nc.dram_tensor("pw", [BL, N], F32, kind="ExternalInput")
    out_d = nc.dram_tensor("out", [BL, N], F32, kind="ExternalOutput")
    cpk_c = nc.inline_tensor(_consts(), "cpk_c")

    with tile.TileContext(nc) as tc:
        with (
            tc.tile_pool(name="const", bufs=1) as cp,
            tc.tile_pool(name="mem", bufs=6) as memp,
            tc.tile_pool(name="sq", bufs=3) as sqp,
            tc.tile_pool(name="light", bufs=1) as lp,
            tc.tile_pool(name="psmm", bufs=2, space="PSUM") as psA,
            tc.tile_pool(name="pstiny", bufs=2, space="PSUM") as psB,
        ):
            # strips memset depends on nothing: issue before everything
            strips = cp.tile([128, BL, 63], BF16)
            nc.vector.memset(strips[:], 0.0)

            # ---- packed input/const DMAs + first stripe quarters ----
            ext_raw = cp.tile([128, EXTW], F32, name="ext_raw")
            nc.sync.dma_start(ext_raw[:], ext_d[:])
            memT_ap = memT_d[:]
            raw_t = []
            st0 = memp.tile([128, N], BF16, tag="raw", name="raw_0")
            raw_t.append(st0)
            Q = N // 4
            for q in range(4):
                nc.sync.dma_start(st0[:, q * Q : (q + 1) * Q],
                                  memT_ap[0][:, q * Q : (q + 1) * Q])
            st1 = memp.tile([128, N], BF16, tag="raw", name="raw_1")
            raw_t.append(st1)
            nc.sync.dma_start(st1[:, 0:HALF], memT_ap[1][:, 0:HALF])
            cpk_raw = cp.tile([128, CPKW], F32, name="cpk_raw")
            nc.sync.dma_start(cpk_raw[:], cpk_c[:])
            nc.sync.dma_start(st1[:, HALF:N], memT_ap[1][:, HALF:N])
            pw_sb = cp.tile([128, W], F32)
            nc.sync.dma_start(pw_sb[:], pw_d[:].rearrange("b (q f) -> (b q) f", f=W))
            for b in range(2, BL):
                st = memp.tile([128, N], BF16, tag="raw", name=f"raw_{b}")
                eng = nc.sync
                eng.dma_start(st[:, 0:HALF], memT_ap[b][:, 0:HALF])
                eng.dma_start(st[:, HALF:N], memT_ap[b][:, HALF:N])
                raw_t.append(st)

            # packed tiles are each written by ONE DMA instruction, so a
            # matmul reading any slices of one pack carries one DMA tick;
            # absorbers below make PE observe each pack's tick once.
            ones_strip = cp.tile([128, 63], BF16, name="strip_g")
            nc.vector.tensor_copy(ones_strip[:], cpk_raw[:, CP_STRIP : CP_STRIP + 63])

            csT0 = ext_raw[:, EXT_CS0 : EXT_CS0 + 128]
            csT1 = ext_raw[:, EXT_CS1 : EXT_CS1 + 128]
            Wk0 = ext_raw[:, EXT_WK0 : EXT_WK0 + 128]
            Wk1 = ext_raw[:, EXT_WK1 : EXT_WK1 + 128]
            Wc0 = ext_raw[:, EXT_WC0 : EXT_WC0 + 6]
            Wc1 = ext_raw[:, EXT_WC1 : EXT_WC1 + 6]
            bias6 = ext_raw[:, EXT_B6 : EXT_B6 + 6]
            ind2 = cpk_raw[:, CP_IND2 : CP_IND2 + 2]
            indT2 = cpk_raw[:, CP_INDT2 : CP_INDT2 + 128]
            pnext = cpk_raw[:, CP_PN : CP_PN + 128]
            pprev = cpk_raw[:, CP_PP : CP_PP + 128]
            indP = cpk_raw[:, CP_IP : CP_IP + 128]
            ind_bb = cpk_raw[:, CP_BB : CP_BB + 128]
            ones_col = cpk_raw[:, CP_ONE : CP_ONE + 1]

            # ---- projections ----
            key_ps = psB.tile([128, 128], F32, tag="keyps", bufs=1)
            nc.tensor.matmul(key_ps[:], lhsT=Wk0, rhs=csT0, start=True, stop=False)
            nc.tensor.matmul(key_ps[:], lhsT=Wk1, rhs=csT1, start=False, stop=True)
            for b in range(BL):
                nc.vector.tensor_copy(
                    strips[:, b, 31:32], key_ps[:, _pos(b) : _pos(b) + 1]
                )

            proj_ps = psB.tile([128, 6], F32, tag="tiny", bufs=3)
            nc.tensor.matmul(proj_ps[:], lhsT=csT0, rhs=Wc0, start=True, stop=False)
            nc.tensor.matmul(proj_ps[:], lhsT=csT1, rhs=Wc1, start=False, stop=True)
            proj = lp.tile([128, 6], F32)
            nc.vector.tensor_add(proj[:], proj_ps[:], bias6)

            # absorbers: make PE observe the cpk-DMA and DVE-strip ticks once
            absorb = psB.tile([128, 8], F32, tag="tiny", bufs=3, name="absorb")
            nc.tensor.matmul(absorb[:, 0:1], lhsT=indT2, rhs=ones_col,
                             start=True, stop=True, skip_group_check=True)
            nc.tensor.matmul(absorb[0:63, 1:2], lhsT=ones_strip[:],
                             rhs=ones_strip[:, 31:32],
                             start=True, stop=True, skip_group_check=True)

            # |key|^2 -> F_kn2 broadcast to window rows
            kq = lp.tile([128, 128], F32)
            nc.scalar.activation(kq[:], key_ps[:], AF.Square)
            kn2_ps = psB.tile([128, 1], F32, tag="tiny", bufs=3)
            nc.tensor.matmul(kn2_ps[:], lhsT=kq[:], rhs=ones_col, start=True, stop=True)
            kn2 = lp.tile([128, 1], F32)
            nc.vector.tensor_copy(kn2[:], kn2_ps[:])
            fkn_ps = psB.tile([128, 1], F32, tag="tiny", bufs=3)
            nc.tensor.matmul(fkn_ps[:], lhsT=indT2, rhs=kn2[:], start=True, stop=True)
            F_kn2 = lp.tile([128, 1], F32)
            nc.vector.tensor_copy(F_kn2[:], fkn_ps[:])

            # ---- per-batch scalars at rows P(b) ----
            scal = lp.tile([128, 7], F32)
            eb = lp.tile([128, 1], F32)
            nc.scalar.activation(eb[:], proj[:, 0:1], AF.Exp)
            sp_b = lp.tile([128, 1], F32)
            nc.scalar.activation(sp_b[:], eb[:], AF.Ln, bias=1.0)
            nc.vector.tensor_scalar_add(scal[:, 0:1], sp_b[:], 1.0)
            eg = lp.tile([128, 1], F32)
            nc.scalar.activation(eg[:], proj[:, 1:2], AF.Exp, scale=-1.0)
            dg = lp.tile([128, 1], F32)
            nc.vector.tensor_scalar_add(dg[:], eg[:], 1.0)
            gate = lp.tile([128, 1], F32)
            nc.vector.reciprocal(gate[:], dg[:])
            nc.vector.tensor_scalar(
                scal[:, 1:2], gate[:], -1.0, 1.0, op0=OP.mult, op1=OP.add
            )
            e3 = lp.tile([128, 3], F32)
            nc.scalar.activation(e3[:], proj[:, 2:5], AF.Exp)
            ssum = lp.tile([128, 1], F32)
            nc.vector.reduce_sum(ssum[:], e3[:], axis=mybir.AxisListType.X)
            rssum = lp.tile([128, 1], F32)
            nc.vector.reciprocal(rssum[:], ssum[:])
            sh3 = lp.tile([128, 3], F32)
            nc.scalar.mul(sh3[:], e3[:], rssum[:])
            nc.vector.tensor_copy(scal[:, 2:5], sh3[:])
            egm = lp.tile([128, 1], F32)
            nc.scalar.activation(egm[:], proj[:, 5:6], AF.Exp)
            sp_g = lp.tile([128, 1], F32)
            nc.scalar.activation(sp_g[:], egm[:], AF.Ln, bias=1.0)
            nc.vector.tensor_scalar_add(scal[:, 5:6], sp_g[:], 1.0)
            nc.vector.tensor_copy(scal[:, 6:7], gate[:])
            FB_ps = psB.tile([128, 7], F32, tag="tiny", bufs=3)
            nc.tensor.matmul(FB_ps[:], lhsT=indT2, rhs=scal[:], start=True, stop=True)
            FB = lp.tile([128, 7], F32)
            nc.vector.tensor_copy(FB[:], FB_ps[:])
            F_beta = FB[:, 0:1]
            F_g1 = FB[:, 1:2]
            F_s0 = FB[:, 2:3]
            F_s1 = FB[:, 3:4]
            F_s2 = FB[:, 4:5]
            F_gamma = FB[:, 5:6]
            F_gate = FB[:, 6:7]

            t4_all = lp.tile([128, W], F32)
            nc.vector.tensor_scalar_mul(t4_all[:], pw_sb[:], F_g1)

            # ---- light tiles ----
            Lv = lp.tile([128, W], F32)
            y1 = lp.tile([128, W], F32)
            sim = lp.tile([128, W], F32)
            E = lp.tile([128, W], F32)
            G = lp.tile([128, W], F32)
            SH = lp.tile([128, W], F32)
            Lg = lp.tile([128, W], F32)
            P2 = lp.tile([128, W], F32)
            outsb = lp.tile([128, W], F32)
            rs1 = lp.tile([128, 1], F32)
            rs2 = lp.tile([128, 1], F32)
            rS = lp.tile([128, 1], F32)
            gs_all = lp.tile([128, 1], F32)
            S2a = lp.tile([128, 1], F32)
            r2a = lp.tile([128, 1], F32)
            F_gs = lp.tile([128, 1], F32)
            F_r2 = lp.tile([128, 1], F32)
            bl_sb = lp.tile([128, 1], F32)
            br_sb = lp.tile([128, 1], F32)

            dotP = psA.tile([128, W], F32, tag="dotP", bufs=1, name="dotP")
            nrmP = psA.tile([128, W], F32, tag="nrmP", bufs=1, name="nrmP")
            sq_t = {}

            def dots(b):
                j = b // 2
                rows = slice(32 * j, 32 * j + 32)
                st = raw_t[b]
                for t in range(NW):
                    c = NW * (b % 2) + t
                    nc.tensor.matmul(
                        dotP[rows, :],
                        lhsT=strips[:, b, 31 - c : 63 - c],
                        rhs=st[:, t * W : (t + 1) * W],
                        start=(b % 2 == 0) and (t == 0),
                        stop=(b % 2 == 1) and (t == NW - 1),
                        skip_group_check=True,
                        tile_position=(0, 32 * j),
                    )

            def squares(b):
                sq = sqp.tile([128, N], BF16, tag="sq", name=f"sq_{b}")
                st = raw_t[b]
                nc.scalar.activation(sq[:, 0:SA], st[:, 0:SA], AF.Square)
                nc.vector.tensor_mul(sq[:, SA:HALF], st[:, SA:HALF], st[:, SA:HALF])
                nc.vector.tensor_mul(sq[:, HALF:SV], st[:, HALF:SV], st[:, HALF:SV])
                nc.gpsimd.tensor_mul(sq[:, SV:N], st[:, SV:N], st[:, SV:N])
                sq_t[b] = sq

            def norms(b):
                j = b // 2
                rows = slice(32 * j, 32 * j + 32)
                sq = sq_t[b]
                for t in range(NW):
                    c = NW * (b % 2) + t
                    nc.tensor.matmul(
                        nrmP[rows, :],
                        lhsT=ones_strip[:, 31 - c : 63 - c],
                        rhs=sq[:, t * W : (t + 1) * W],
                        start=(b % 2 == 0) and (t == 0),
                        stop=(b % 2 == 1) and (t == NW - 1),
                        skip_group_check=True,
                        tile_position=(0, 32 * j),
                    )

            def stageA(j):
                R = slice(32 * j, 32 * j + 32)
                nc.scalar.activation(Lv[R, :], nrmP[R, :], AF.Ln,
                                     scale=F_kn2[R, :])
                nc.scalar.activation(y1[R, :], Lv[R, :], AF.Exp, scale=-0.5)
                nc.vector.tensor_mul(sim[R, :], dotP[R, :], y1[R, :])
                nc.scalar.activation(E[R, :], sim[R, :], AF.Exp,
                                     scale=F_beta[R, :], accum_out=rs1[R, :])

            for it in range(BL + 2):
                if it < BL:
                    dots(it)
                    squares(it)
                if it >= 2:
                    norms(it - 2)
                    if (it - 2) % 2 == 1:
                        stageA((it - 2) // 2)

            # ---- tail: batch-sum-broadcast via block-diag ind_bb in ONE
            # matmul, then full-width chain ----
            sps = psB.tile([128, 1], F32, tag="tiny", bufs=3, name="sps")
            nc.tensor.matmul(sps[:], lhsT=ind_bb, rhs=rs1[:],
                             start=True, stop=True, skip_group_check=True)
            nc.vector.reciprocal(rS[:], sps[:])
            nc.vector.tensor_mul(F_gs[:], F_gate, rS[:])
            nc.vector.scalar_tensor_tensor(
                G[:], E[:], F_gs[:], t4_all[:], op0=OP.mult, op1=OP.add
            )
            blps = psB.tile([128, 1], F32, tag="tiny", bufs=3, name="blps")
            nc.tensor.matmul(blps[:], lhsT=pnext, rhs=G[:, 0:1],
                             start=True, stop=True, skip_group_check=True)
            brps = psB.tile([128, 1], F32, tag="tiny", bufs=3, name="brps")
            nc.tensor.matmul(brps[:], lhsT=pprev, rhs=G[:, W - 1 : W],
                             start=True, stop=True, skip_group_check=True)
            nc.vector.tensor_copy(bl_sb[:], blps[:])
            nc.vector.tensor_copy(br_sb[:], brps[:])
            nc.scalar.mul(SH[:], G[:], F_s1)
            nc.vector.scalar_tensor_tensor(
                SH[:, 0 : W - 1], G[:, 1:W], F_s0, SH[:, 0 : W - 1],
                op0=OP.mult, op1=OP.add,
            )
            nc.vector.scalar_tensor_tensor(
                SH[:, 1:W], G[:, 0 : W - 1], F_s2, SH[:, 1:W],
                op0=OP.mult, op1=OP.add,
            )
            nc.vector.scalar_tensor_tensor(
                SH[:, W - 1 : W], bl_sb[:], F_s0, SH[:, W - 1 : W],
                op0=OP.mult, op1=OP.add,
            )
            nc.vector.scalar_tensor_tensor(
                SH[:, 0:1], br_sb[:], F_s2, SH[:, 0:1],
                op0=OP.mult, op1=OP.add,
            )
            nc.scalar.activation(Lg[:], SH[:], AF.Ln,
                                 bias=cpk_raw[:, CP_EPS : CP_EPS + 1])
            nc.scalar.activation(P2[:], Lg[:], AF.Exp,
                                 scale=F_gamma, accum_out=rs2[:])
            s2ps = psB.tile([128, 1], F32, tag="tiny", bufs=3, name="s2ps")
            nc.tensor.matmul(s2ps[:], lhsT=ind_bb, rhs=rs2[:],
                             start=True, stop=True, skip_group_check=True)
            nc.vector.tensor_scalar_add(S2a[:], s2ps[:], EPS)
            nc.vector.reciprocal(F_r2[:], S2a[:])
            nc.scalar.mul(outsb[:], P2[:], F_r2)
            nc.sync.dma_start(
                out_d[:].rearrange("b (q f) -> (b q) f", f=W), outsb[:]
            )
    nc.compile()
    return nc


def _get_nc():
    global _NC
    if _NC is None:
        _NC = build_nc()
    return _NC


def _enable_profiling():
    """Install the axon NTFF profile hook; the agent image lacks
    antenv.axon_hooks, so shim it and register the ctypes-based hook."""
    import types

    import concourse.bass_utils as bu

    bu.upload_artifacts = lambda tmpdir: tmpdir
    try:
        from antenv.axon_hooks import get_axon_ntff_profile_hook  # noqa: F401

        return
    except ImportError:
        pass
    import antenv

    mod = types.ModuleType("antenv.axon_hooks")
    _holder = {}
    mod.set_axon_ntff_profile_hook = lambda h: _holder.__setitem__("h", h)
    mod.get_axon_ntff_profile_hook = lambda: _holder.get("h")
    sys.modules["antenv.axon_hooks"] = mod
    antenv.axon_hooks = mod
    from trn_agent_boot.trn_boot import _ntff_profile_via_ctypes

    mod.set_axon_ntff_profile_hook(
        _ntff_profile_via_ctypes("/opt/axon/libaxon_pjrt.so")
    )


def kernel(**inputs):
    global LAST_RESULTS
    mem = np.ascontiguousarray(np.asarray(inputs["memory"], dtype=np.float32))
    cs = np.ascontiguousarray(np.asarray(inputs["controller_state"], dtype=np.float32))
    pw = np.ascontiguousarray(np.asarray(inputs["previous_weights"], dtype=np.float32))
    Wk = np.ascontiguousarray(np.asarray(inputs["Wk"], dtype=np.float32))
    Wcat = np.concatenate(
        [
            np.asarray(inputs["Wb"], np.float32),
            np.asarray(inputs["Wg"], np.float32),
            np.asarray(inputs["Ws"], np.float32),
            np.asarray(inputs["Wgam"], np.float32),
        ],
        axis=1,
    )
    brow = np.concatenate(
        [
            np.asarray(inputs["bb"], np.float32),
            np.asarray(inputs["bg"], np.float32),
            np.asarray(inputs["bs"], np.float32),
            np.asarray(inputs["bgam"], np.float32),
        ]
    )
    pos = np.array([_pos(b) for b in range(BL)])

    # shard: core c gets batches [c*BL, (c+1)*BL); memory pre-transposed
    memT = np.ascontiguousarray(
        mem.reshape(NCORES, BL, N, D).transpose(0, 1, 3, 2)
    )
    import ml_dtypes
    memT = memT.astype(ml_dtypes.bfloat16)
    csT_all = cs.reshape(NCORES, BL, C).transpose(0, 2, 1)  # (8, C, BL)
    pw_sh = pw.reshape(NCORES, BL, N)

    # packed external input: csT0|csT1|Wk0|Wk1|Wc0|Wc1|bias6
    ext = np.zeros((NCORES, 128, EXTW), np.float32)
    ext[:, :, EXT_WK0 : EXT_WK0 + 128] = Wk[0:128, :][None]
    ext[:, :, EXT_WK1 : EXT_WK1 + 128] = Wk[128:256, :][None]
    ext[:, :, EXT_WC0 : EXT_WC0 + 6] = Wcat[0:128, :][None]
    ext[:, :, EXT_WC1 : EXT_WC1 + 6] = Wcat[128:256, :][None]
    ext[:, pos, EXT_B6 : EXT_B6 + 6] = brow[None, None, :]
    for bi in range(BL):
        ext[:, :, EXT_CS0 + pos[bi]] = csT_all[:, 0:128, bi]
        ext[:, :, EXT_CS1 + pos[bi]] = csT_all[:, 128:256, bi]

    in_maps = [
        {
            "memT": memT[c],
            "ext": np.ascontiguousarray(ext[c]),
            "pw": np.ascontiguousarray(pw_sh[c]),
        }
        for c in range(NCORES)
    ]
    nc = _get_nc()
    if PROFILE:
        _enable_profiling()
    res = run_bass_kernel_spmd(nc, in_maps, list(range(NCORES)), trace=PROFILE)
    LAST_RESULTS = res
    out = np.concatenate([r["out"] for r in res.results], axis=0)
    return out.astype(np.float32)
